# revision 20
# baseline (speedup 1.0000x reference)
"""MiniBatchDiscrimination Trainium2 kernel — hw-sharded symmetric version.

reference:
    M = einsum('nhwf,fbc->nhwbc', x, T)          # [N,H,W,B,C]
    norm = sum_c |M[i] - M[j]|                   # [N,N,H,W,B]
    o_b  = sum_j exp(-norm)                      # [N,H,W,B]
    out  = concat([x, o_b], axis=3)              # [N,H,W,F+B]

Sharding: the whole computation is embarrassingly parallel over the HW=256
spatial positions, so each core takes a 32-position hw slice and computes
ALL pairs for it — no replicated M compute and no cross-core traffic.

Pair symmetry: |M_i - M_j| is symmetric, so each unordered pair is computed
once via offset streams d=1..16: stream d covers pairs (i, (i+d)%32) for all
i (d=16: i<16 only).  Each stream's exp(-norm) is accumulated into BOTH row
i (direct) and row (i+d)%32 (shifted) by TensorEngine ones-matmuls into a
single PSUM accumulator; the diagonal contributes a constant +1 fused into
the final PSUM drain.

Per-core layout:
  M2[q]   [128 part=(b16,c8), 1024 free=(i32,hwl32)] f16, per b-quarter q
  ad      [128, 1024] f16  |M2[:, i+d] - M2[:, i]|   (DVE custom op, 2X)
  nrm     [128 part=(dp2,q4,b16), 1024 free=(i,hwl)] f32 PSUM — c-reduce via
          col-tiled (128x32) stripe-ones matmuls, 2 d-slots per tile
  E       exp(-nrm) f16 (ACT)
  o_ps    [64 part=(q4,b16), 1024 free=(i,hwl)] f32 PSUM — j-sum via
          col-tiled fold matmuls (direct + d-shifted reads of E)
"""

import os
import sys

for _p in ("/opt/trn_rl_repo", "/opt/pypackages"):
    if _p not in sys.path and os.path.isdir(_p):
        sys.path.append(_p)

import numpy as np

N, HWL, F, B, C = 32, 32, 256, 64, 8
HW = 256
CORES = 8
FH = 2          # f in two partition halves of 128
Q = 4           # b-quarters of 16

F16 = "float16"


def _absdiff_uop_1x():
    """REGULAR program: |a-b| via SUB, reverse-SUB, MAX on slices 0-2."""
    from concourse.dve_uop import (
        ENABLE, AluInp, AluOp, DelayInp, InpSel, OutPath, OutSel, Trigger,
        UopConfig, UopDpConfig,
    )

    u = UopConfig()
    u.enable_input(InpSel.SRC_0, 0).enable_input(InpSel.SRC_1, 1)
    u.require_inp0 = ENABLE
    u.require_inp1 = ENABLE
    u.trigger = (Trigger.SRC_TENSOR_DONE, Trigger.NONE, Trigger.NONE)
    u.enable_output(OutSel.ALU_OUT, OutPath.WR0_LO)
    dp = u.datapath_config
    # s0: alu = a - b; carry b (chain0), capture a (chain3)
    dp[0] = (UopDpConfig()
             .enable_alu(AluOp.SUBTRACT, AluInp.PREV_ALU_OUT, AluInp.PREV_DELAY_0)
             .pass_through_delay(0)
             .enable_delay_from_src(DelayInp.PREV_ALU_OUT, 3))
    # s1: alu = b - a; capture (a-b) into chain0
    dp[1] = (UopDpConfig()
             .enable_alu(AluOp.SUBTRACT, AluInp.PREV_DELAY_0, AluInp.PREV_DELAY_3)
             .enable_delay_from_src(DelayInp.PREV_ALU_OUT, 0))
    # s2: alu = max(b-a, a-b)
    dp[2] = UopDpConfig().enable_alu(
        AluOp.MAX, AluInp.PREV_ALU_OUT, AluInp.PREV_DELAY_0)
    for i in range(3, 8):
        dp[i] = UopDpConfig().pass_through_alu()
    return u


def _absdiff_uop_2x():
    """2X_1PORT program: lo on slices 0-2, hi on slices 3-5."""
    from concourse.dve_uop import (
        ENABLE, AluInp, AluOp, DelayInp, InpSel, OutPath, OutSel, Trigger,
        UopConfig, UopDpConfig,
    )

    u = UopConfig()
    u.enable_input(InpSel.SRC_0, 0).enable_input(InpSel.SRC_1, 1)
    u.enable_input(InpSel.SRC_0_HI, 2).enable_input(InpSel.SRC_1_HI, 3)
    u.require_inp0 = ENABLE
    u.require_inp1 = ENABLE
    u.trigger = (Trigger.SRC_TENSOR_DONE, Trigger.NONE, Trigger.NONE)
    u.enable_output(OutSel.DELAY_0, OutPath.WR0_LO)   # lo result rides chain0
    u.enable_output(OutSel.ALU_OUT, OutPath.WR0_HI)   # hi result on ALU lane
    dp = u.datapath_config
    # s0: alu = a_lo - b_lo; carry b_lo(c0), a_hi(c1), b_hi(c2); capture a_lo(c3)
    dp[0] = (UopDpConfig()
             .enable_alu(AluOp.SUBTRACT, AluInp.PREV_ALU_OUT, AluInp.PREV_DELAY_0)
             .pass_through_delay(0, 1, 2)
             .enable_delay_from_src(DelayInp.PREV_ALU_OUT, 3))
    # s1: alu = b_lo - a_lo; capture (a-b)_lo into c0; carry a_hi, b_hi
    dp[1] = (UopDpConfig()
             .enable_alu(AluOp.SUBTRACT, AluInp.PREV_DELAY_0, AluInp.PREV_DELAY_3)
             .enable_delay_from_src(DelayInp.PREV_ALU_OUT, 0)
             .pass_through_delay(1, 2))
    # s2: alu = max -> |a-b|_lo; carry a_hi, b_hi
    dp[2] = (UopDpConfig()
             .enable_alu(AluOp.MAX, AluInp.PREV_ALU_OUT, AluInp.PREV_DELAY_0)
             .pass_through_delay(1, 2))
    # s3: alu = a_hi - b_hi; capture lo result into c0; carry a_hi, b_hi
    dp[3] = (UopDpConfig()
             .enable_alu(AluOp.SUBTRACT, AluInp.PREV_DELAY_1, AluInp.PREV_DELAY_2)
             .enable_delay_from_src(DelayInp.PREV_ALU_OUT, 0)
             .pass_through_delay(1, 2))
    # s4: alu = b_hi - a_hi; carry lo(c0); capture (a-b)_hi into c3
    dp[4] = (UopDpConfig()
             .enable_alu(AluOp.SUBTRACT, AluInp.PREV_DELAY_2, AluInp.PREV_DELAY_1)
             .pass_through_delay(0)
             .enable_delay_from_src(DelayInp.PREV_ALU_OUT, 3))
    # s5: alu = max -> |a-b|_hi; carry lo(c0)
    dp[5] = (UopDpConfig()
             .enable_alu(AluOp.MAX, AluInp.PREV_ALU_OUT, AluInp.PREV_DELAY_3)
             .pass_through_delay(0))
    # s6, s7: pass alu (hi) + chain0 (lo)
    for i in (6, 7):
        dp[i] = UopDpConfig().pass_through_alu().pass_through_delay(0)
    return u


def _get_absdiff_op():
    """Fused |a-b| custom DVE op with a hand-written 2X_1PORT variant."""
    if "absdiff" in _CACHED:
        return _CACHED["absdiff"]
    from concourse import dve_ops
    from concourse.dve_spec import Spec, Src0, Src1, maxx
    from concourse.dve_uop import DveOpSpec

    NAME = "ABSDIFF_ANT"
    for op in dve_ops.OPS:
        if op.name == NAME:
            _CACHED["absdiff"] = op
            return op
    spec = Spec(
        body=maxx(Src0 - Src1, Src1 - Src0),
        reference=lambda in0, in1, s0, s1, imm2: np.abs(
            in0.astype(np.float32) - in1.astype(np.float32)
        ),
    )
    op = dve_ops.DveOp(NAME, spec, subdim=False, uops_sha={})
    dve_ops.OPS.append(op)
    dve_ops.CUSTOM_DVE_SPECS[op.name] = op.spec
    row = dve_ops._CUSTOM_DVE_ROW_BASE + len(dve_ops.OPS) - 1
    dve_ops._SUB_OPCODE_FOR_NAME[op.name] = row
    compiled = DveOpSpec(
        name=NAME,
        opcode=row,
        uops=[_absdiff_uop_1x()],
        uops_2x=[_absdiff_uop_2x()],
        perf_max=1,
        rd1_en=True,
    )
    compiled.validate("v3")
    dve_ops._COMPILE_CACHE[(NAME, "v3")] = compiled
    dve_ops._COMPILE_CACHE[(NAME, "v4")] = compiled
    _CACHED["absdiff"] = op
    return op


# --------------------------------------------------------------------------
# device program
# --------------------------------------------------------------------------

def make_pools(tc, ctx, rep=0):
    sfx = f"_{rep}"
    singles = ctx.enter_context(tc.tile_pool(name="singles" + sfx, bufs=1))
    dbl = ctx.enter_context(tc.tile_pool(name="dbl" + sfx, bufs=2))
    psA = ctx.enter_context(tc.tile_pool(name="psA" + sfx, bufs=2, space="PSUM"))
    psN = ctx.enter_context(tc.tile_pool(name="psN" + sfx, bufs=1, space="PSUM"))
    psO = ctx.enter_context(tc.tile_pool(name="psO" + sfx, bufs=1, space="PSUM"))
    adp = ctx.enter_context(tc.tile_pool(name="adp" + sfx, bufs=6))
    Ep = ctx.enter_context(tc.tile_pool(name="Ep" + sfx, bufs=3))
    return singles, dbl, psA, psN, psO, adp, Ep


def build_body(tc, outs, ins, rep=0, pools=None):
    """Trace the per-core Tile program.

    ins:  xT   [2,128,1024] f16  xT[fh,f,i*32+hwl] = x[i, hw(core,hwl), fh*128+f]
          tw   [2,4,128,128] f16 tw[fh,q,f,b*8+c] = T[fh*128+f,16q+b,c]
          wts  [14,128,32]  f16  0-7: stripe-ones (c-reduce), 8-9: direct
                                 folds per col group, 10-13: half folds (dp,gp)
    outs: o    [64,1024]    f32  o[16q+b, i*32+hwl] = o_b[i, hw(core,hwl), 16q+b]
    """
    from contextlib import ExitStack

    import concourse.mybir as mybir

    nc = tc.nc
    f16 = mybir.dt.float16
    f32 = mybir.dt.float32

    xT_d, tw_d, wts_d = ins["xT"], ins["tw"], ins["wts"]
    o_d = outs["o"]

    with ExitStack() as ctx:
        if pools is None:
            pools = make_pools(tc, ctx, rep)
        singles, dbl, psA, psN, psO, adp, Ep = pools

        # ---- loads (host packs partition-first); tw first and xT split per
        # f-half so stage B's first matmul starts as early as possible.
        tw_t = singles.tile([128, FH * Q * 128], f16, tag="tw")
        nc.sync.dma_start(out=tw_t, in_=tw_d)
        tw_s = [[tw_t[:, (fh * Q + q) * 128:(fh * Q + q + 1) * 128]
                 for q in range(Q)] for fh in range(FH)]
        xT_t = dbl.tile([128, FH * N * HWL], f16, tag="xT")
        nc.sync.dma_start(out=xT_t[:, 0:1024], in_=xT_d[:, 0:1024])
        nc.sync.dma_start(out=xT_t[:, 1024:2048], in_=xT_d[:, 1024:2048])
        xT_s = [xT_t[:, fh * 1024:(fh + 1) * 1024] for fh in range(FH)]
        wts_t = singles.tile([128, 14 * 32], f16, tag="wts")
        nc.sync.dma_start(out=wts_t, in_=wts_d)
        wts_s = [wts_t[:, w * 32:(w + 1) * 32] for w in range(14)]
        ones_s = wts_s[0:8]
        fold_s = wts_s[8:10]            # fold_s[gp]
        hfold_s = [wts_s[10:12], wts_s[12:14]]   # hfold_s[dp][gp]

        # ---- stage B: M2 = (x_slice @ T_q), (b,c)-partition layout, with
        # 512 circularly-padded columns per quarter so every d-stream is one
        # contiguous read: M2v[p, q, k] for k in [0,1536), k>=1024 wraps.
        m2all = dbl.tile([128, Q * 1536], f16, tag="m2")
        M2v = m2all.rearrange("p (q x) -> p q x", q=Q)
        for q in range(Q):
            ps = psA.tile([128, 1024], f32, tag="psA")
            for fh in range(FH):
                for sub in range(2):
                    sl = slice(sub * 512, (sub + 1) * 512)
                    nc.tensor.matmul(
                        ps[:, sl], lhsT=tw_s[fh][q], rhs=xT_s[fh][:, sl],
                        start=(fh == 0), stop=(fh == 1),
                    )
            nc.scalar.copy(out=m2all[:, q * 1536:q * 1536 + 1024], in_=ps[:])
            nc.scalar.copy(out=m2all[:, q * 1536 + 1024:(q + 1) * 1536],
                           in_=ps[:, 0:512])

        # ---- stage C: d-streams ------------------------------------------
        o_ps = psO.tile([128, 1024], f32, tag="oPs")   # rows 0-63 used
        for t in range(8):
            # absdiff for the two d-slots of this group.  Early groups go
            # per-quarter so the DVE starts as soon as stage B's first
            # quarter lands; later groups use one 3D-AP instruction per d.
            ads2 = []
            if t < 2:
                ad0 = adp.tile([128, Q * 1024], f16, tag="ad")
                ad1 = adp.tile([128, Q * 1024], f16, tag="ad")
                ads2 = [ad0, ad1]
                for q in range(Q):
                    for dp in range(2):
                        d = 2 * t + 1 + dp
                        bi = nc.vector._custom_dve(
                            _get_absdiff_op(),
                            out=ads2[dp][:, q * 1024:(q + 1) * 1024],
                            in0=M2v[:, q, d * 32:d * 32 + 1024],
                            in1=M2v[:, q, 0:1024],
                        )
                        bi.ins.perf_max = 1
            else:
                for dp in range(2):
                    d = 2 * t + 1 + dp
                    ad = adp.tile([128, Q * 1024], f16, tag="ad")
                    adv = ad.rearrange("p (q x) -> p q x", q=Q)
                    ln = 1024 if d < 16 else 512
                    bi = nc.vector._custom_dve(
                        _get_absdiff_op(), out=adv[:, :, 0:ln],
                        in0=M2v[:, :, d * 32:d * 32 + ln], in1=M2v[:, :, 0:ln],
                    )
                    bi.ins.perf_max = 1
                    ads2.append(ad)

            # c-reduce: col-tiled stripe-ones matmuls, stripe s = 4*dp+q ->
            # col group s//2, partitions 16s+b = (dp, q, b)
            nrm = psN.tile([128, 1024], f32, tag="nrm")
            for sp in range(2):
                for h in range(2):
                    for g in range(4):
                        s = 2 * g + sp
                        dp, q = s // 4, s % 4
                        hs = slice(h * 512, (h + 1) * 512)
                        nc.tensor.matmul(
                            nrm[32 * g:32 * g + 32, hs],
                            lhsT=ones_s[s],
                            rhs=ads2[dp][:, q * 1024 + h * 512:
                                          q * 1024 + (h + 1) * 512],
                            start=(sp == 0), stop=(sp == 1),
                            tile_position=(0, 32 * g),
                        )

            # exp(-nrm) -> f16; the last group splits by half and leaves the
            # unused (i>=16) region of the d=16 slot to an early memset so
            # nothing serializes the tail.
            E = Ep.tile([128, 1024], f16, tag="E")
            if t == 7:
                nc.vector.memset(E[64:128, 512:1024], 0.0)
                nc.scalar.activation(
                    out=E[:, 0:512], in_=nrm[:, 0:512],
                    func=mybir.ActivationFunctionType.Exp, scale=-1.0,
                )
                nc.scalar.activation(
                    out=E[0:64, 512:1024], in_=nrm[0:64, 512:1024],
                    func=mybir.ActivationFunctionType.Exp, scale=-1.0,
                )
            else:
                nc.scalar.activation(
                    out=E, in_=nrm[:],
                    func=mybir.ActivationFunctionType.Exp, scale=-1.0,
                )

            # j-sum: direct (row i) + shifted (row i+d) accumulation
            for h in range(2):
                for gp in range(2):
                    hs = slice(h * 512, (h + 1) * 512)
                    nc.tensor.matmul(
                        o_ps[32 * gp:32 * gp + 32, hs],
                        lhsT=fold_s[gp], rhs=E[:, hs],
                        start=(t == 0), stop=False,
                        tile_position=(0, 32 * gp),
                        skip_group_check=True,
                    )
            for dp in range(2):
                d = 2 * t + 1 + dp
                if d == 16:
                    segs = [(512, 1024, 0)]
                else:
                    segs = [
                        (32 * d, 512, 0),
                        (512, 1024, 512 - 32 * d),
                        (0, 32 * d, 1024 - 32 * d),
                    ]
                for si, (o0, o1, r0) in enumerate(segs):
                    ln = o1 - o0
                    last = (t == 7 and dp == 1 and si == len(segs) - 1)
                    for gp in range(2):
                        nc.tensor.matmul(
                            o_ps[32 * gp:32 * gp + 32, o0:o1],
                            lhsT=hfold_s[dp][gp], rhs=E[:, r0:r0 + ln],
                            start=False, stop=last,
                            tile_position=(0, 32 * gp),
                            skip_group_check=True,
                        )

        # ---- diagonal (+1) fused into the PSUM drain, then DMA out.  Done
        # per 512-column half so bank A drains while PE finishes bank B.
        o_sb = singles.tile([64, 1024], f32, tag="osb")
        for h in range(2):
            hs = slice(h * 512, (h + 1) * 512)
            nc.scalar.activation(
                out=o_sb[:, hs], in_=o_ps[0:64, hs],
                func=mybir.ActivationFunctionType.Identity, bias=1.0, scale=1.0,
            )
            nc.sync.dma_start(out=o_d[:, hs], in_=o_sb[:, hs])


# --------------------------------------------------------------------------
# host side
# --------------------------------------------------------------------------

def prep_inputs(x, T):
    """Shared (core-independent) device inputs, packed partition-first."""
    xf = np.ascontiguousarray(x.reshape(N, HW, F))
    tw = T.reshape(FH, 128, Q, 16, C).transpose(0, 2, 1, 3, 4)
    tw_in = tw.reshape(FH, Q, 128, 128)
    tw_in = np.ascontiguousarray(
        tw_in.transpose(2, 0, 1, 3).reshape(128, FH * Q * 128)
    ).astype(np.float16)
    wts_in = np.zeros((14, 128, 32), np.float16)
    for s in range(8):
        for b in range(16):
            wts_in[s, b * 8:(b + 1) * 8, 16 * (s % 2) + b] = 1.0
    for gp in range(2):
        for dp in range(2):
            for q in (2 * gp, 2 * gp + 1):
                for b in range(16):
                    col = 16 * (q - 2 * gp) + b
                    wts_in[8 + gp, 64 * dp + 16 * q + b, col] = 1.0
                    wts_in[10 + 2 * dp + gp, 64 * dp + 16 * q + b, col] = 1.0
    wts_in = np.ascontiguousarray(
        wts_in.transpose(1, 0, 2).reshape(128, 14 * 32))
    return xf, tw_in, wts_in


def core_in_map(xf, tw_in, wts_in, k):
    xs = xf[:, k * HWL:(k + 1) * HWL, :]          # [i, hwl, f]
    xT = xs.transpose(2, 0, 1).reshape(FH, 128, N * HWL)
    xT = np.ascontiguousarray(xT.transpose(1, 0, 2).reshape(128, FH * N * HWL))
    return {"xT": xT.astype(np.float16), "tw": tw_in, "wts": wts_in}


def gather_ob(core_outs):
    """core_outs: list of 8 arrays [64,1024] f32 -> o_b [N,16,16,B]."""
    obs = []
    for res in core_outs:
        v = res.astype(np.float32).reshape(B, N, HWL)   # (16q+b), i, hwl
        obs.append(v.transpose(1, 2, 0))                # i, hwl, b
    return np.concatenate(obs, axis=1).reshape(N, 16, 16, B)


_CACHED = {}


def _get_program(reps=1, loop=None):
    key = ("nc", reps, loop)
    if key in _CACHED:
        return _CACHED[key]
    from contextlib import ExitStack
    import concourse.bacc as bacc
    import concourse.mybir as mybir
    import concourse.tile as tile

    nc = bacc.Bacc("TRN2", target_bir_lowering=False, debug=False,
                   num_devices=CORES)
    f16, f32 = mybir.dt.float16, mybir.dt.float32
    ins = {
        "xT": nc.dram_tensor("xT", [128, FH * N * HWL], f16,
                             kind="ExternalInput").ap(),
        "tw": nc.dram_tensor("tw", [128, FH * Q * 128], f16,
                             kind="ExternalInput").ap(),
        "wts": nc.dram_tensor("wts", [128, 14 * 32], f16,
                              kind="ExternalInput").ap(),
    }
    outs = {
        "o": nc.dram_tensor("o", [64, N * HWL], f32, kind="ExternalOutput").ap(),
    }
    with tile.TileContext(nc) as tc:
        if loop:
            with ExitStack() as ctx:
                pools = make_pools(tc, ctx)
                with tc.For_i(0, loop, 1,
                              hint_engines=(mybir.EngineType.PE,
                                            mybir.EngineType.DVE)):
                    build_body(tc, outs, ins, pools=pools)
        else:
            for r in range(reps):
                build_body(tc, outs, ins, rep=r)
    nc.compile()
    _CACHED[key] = nc
    return nc


def kernel(x, T):
    x = np.asarray(x, dtype=np.float32)
    T = np.asarray(T, dtype=np.float32)
    from concourse.bass_utils import run_bass_kernel_spmd

    nc = _get_program()
    xf, tw_in, wts_in = prep_inputs(x, T)
    in_maps = [core_in_map(xf, tw_in, wts_in, k) for k in range(CORES)]
    res = run_bass_kernel_spmd(nc, in_maps, core_ids=list(range(CORES)))
    ob = gather_ob([r["o"] for r in res.results])
    return np.concatenate([x, ob], axis=3)


# revision 25
# speedup vs baseline: 1.0754x; 1.0754x over previous
"""MiniBatchDiscrimination Trainium2 kernel — hw-sharded symmetric version.

reference:
    M = einsum('nhwf,fbc->nhwbc', x, T)          # [N,H,W,B,C]
    norm = sum_c |M[i] - M[j]|                   # [N,N,H,W,B]
    o_b  = sum_j exp(-norm)                      # [N,H,W,B]
    out  = concat([x, o_b], axis=3)              # [N,H,W,F+B]

Sharding: the whole computation is embarrassingly parallel over the HW=256
spatial positions, so each core takes a 32-position hw slice and computes
ALL pairs for it — no replicated M compute and no cross-core traffic.

Pair symmetry: |M_i - M_j| is symmetric, so each unordered pair is computed
once via offset streams d=1..16: stream d covers pairs (i, (i+d)%32) for all
i (d=16: i<16 only).  Each stream's exp(-norm) is accumulated into BOTH row
i (direct) and row (i+d)%32 (shifted) by TensorEngine ones-matmuls into a
single PSUM accumulator; the diagonal contributes a constant +1 fused into
the final PSUM drain.

Per-core layout:
  M2[q]   [128 part=(b16,c8), 1024 free=(i32,hwl32)] f16, per b-quarter q
  ad      [128, 1024] f16  |M2[:, i+d] - M2[:, i]|   (DVE custom op, 2X)
  nrm     [128 part=(dp2,q4,b16), 1024 free=(i,hwl)] f32 PSUM — c-reduce via
          col-tiled (128x32) stripe-ones matmuls, 2 d-slots per tile
  E       exp(-nrm) f16 (ACT)
  o_ps    [64 part=(q4,b16), 1024 free=(i,hwl)] f32 PSUM — j-sum via
          col-tiled fold matmuls (direct + d-shifted reads of E)
"""

import os
import sys

for _p in ("/opt/trn_rl_repo", "/opt/pypackages"):
    if _p not in sys.path and os.path.isdir(_p):
        sys.path.append(_p)

import numpy as np

N, HWL, F, B, C = 32, 32, 256, 64, 8
HW = 256
CORES = 8
FH = 2          # f in two partition halves of 128
Q = 4           # b-quarters of 16

F16 = "float16"


def _absdiff_uop_1x():
    """REGULAR program: |a-b| via SUB, reverse-SUB, MAX on slices 0-2."""
    from concourse.dve_uop import (
        ENABLE, AluInp, AluOp, DelayInp, InpSel, OutPath, OutSel, Trigger,
        UopConfig, UopDpConfig,
    )

    u = UopConfig()
    u.enable_input(InpSel.SRC_0, 0).enable_input(InpSel.SRC_1, 1)
    u.require_inp0 = ENABLE
    u.require_inp1 = ENABLE
    u.trigger = (Trigger.SRC_TENSOR_DONE, Trigger.NONE, Trigger.NONE)
    u.enable_output(OutSel.ALU_OUT, OutPath.WR0_LO)
    dp = u.datapath_config
    # s0: alu = a - b; carry b (chain0), capture a (chain3)
    dp[0] = (UopDpConfig()
             .enable_alu(AluOp.SUBTRACT, AluInp.PREV_ALU_OUT, AluInp.PREV_DELAY_0)
             .pass_through_delay(0)
             .enable_delay_from_src(DelayInp.PREV_ALU_OUT, 3))
    # s1: alu = b - a; capture (a-b) into chain0
    dp[1] = (UopDpConfig()
             .enable_alu(AluOp.SUBTRACT, AluInp.PREV_DELAY_0, AluInp.PREV_DELAY_3)
             .enable_delay_from_src(DelayInp.PREV_ALU_OUT, 0))
    # s2: alu = max(b-a, a-b)
    dp[2] = UopDpConfig().enable_alu(
        AluOp.MAX, AluInp.PREV_ALU_OUT, AluInp.PREV_DELAY_0)
    for i in range(3, 8):
        dp[i] = UopDpConfig().pass_through_alu()
    return u


def _absdiff_uop_2x():
    """2X_1PORT program: lo on slices 0-2, hi on slices 3-5."""
    from concourse.dve_uop import (
        ENABLE, AluInp, AluOp, DelayInp, InpSel, OutPath, OutSel, Trigger,
        UopConfig, UopDpConfig,
    )

    u = UopConfig()
    u.enable_input(InpSel.SRC_0, 0).enable_input(InpSel.SRC_1, 1)
    u.enable_input(InpSel.SRC_0_HI, 2).enable_input(InpSel.SRC_1_HI, 3)
    u.require_inp0 = ENABLE
    u.require_inp1 = ENABLE
    u.trigger = (Trigger.SRC_TENSOR_DONE, Trigger.NONE, Trigger.NONE)
    u.enable_output(OutSel.DELAY_0, OutPath.WR0_LO)   # lo result rides chain0
    u.enable_output(OutSel.ALU_OUT, OutPath.WR0_HI)   # hi result on ALU lane
    dp = u.datapath_config
    # s0: alu = a_lo - b_lo; carry b_lo(c0), a_hi(c1), b_hi(c2); capture a_lo(c3)
    dp[0] = (UopDpConfig()
             .enable_alu(AluOp.SUBTRACT, AluInp.PREV_ALU_OUT, AluInp.PREV_DELAY_0)
             .pass_through_delay(0, 1, 2)
             .enable_delay_from_src(DelayInp.PREV_ALU_OUT, 3))
    # s1: alu = b_lo - a_lo; capture (a-b)_lo into c0; carry a_hi, b_hi
    dp[1] = (UopDpConfig()
             .enable_alu(AluOp.SUBTRACT, AluInp.PREV_DELAY_0, AluInp.PREV_DELAY_3)
             .enable_delay_from_src(DelayInp.PREV_ALU_OUT, 0)
             .pass_through_delay(1, 2))
    # s2: alu = max -> |a-b|_lo; carry a_hi, b_hi
    dp[2] = (UopDpConfig()
             .enable_alu(AluOp.MAX, AluInp.PREV_ALU_OUT, AluInp.PREV_DELAY_0)
             .pass_through_delay(1, 2))
    # s3: alu = a_hi - b_hi; capture lo result into c0; carry a_hi, b_hi
    dp[3] = (UopDpConfig()
             .enable_alu(AluOp.SUBTRACT, AluInp.PREV_DELAY_1, AluInp.PREV_DELAY_2)
             .enable_delay_from_src(DelayInp.PREV_ALU_OUT, 0)
             .pass_through_delay(1, 2))
    # s4: alu = b_hi - a_hi; carry lo(c0); capture (a-b)_hi into c3
    dp[4] = (UopDpConfig()
             .enable_alu(AluOp.SUBTRACT, AluInp.PREV_DELAY_2, AluInp.PREV_DELAY_1)
             .pass_through_delay(0)
             .enable_delay_from_src(DelayInp.PREV_ALU_OUT, 3))
    # s5: alu = max -> |a-b|_hi; carry lo(c0)
    dp[5] = (UopDpConfig()
             .enable_alu(AluOp.MAX, AluInp.PREV_ALU_OUT, AluInp.PREV_DELAY_3)
             .pass_through_delay(0))
    # s6, s7: pass alu (hi) + chain0 (lo)
    for i in (6, 7):
        dp[i] = UopDpConfig().pass_through_alu().pass_through_delay(0)
    return u


def _get_absdiff_op():
    """Fused |a-b| custom DVE op with a hand-written 2X_1PORT variant."""
    if "absdiff" in _CACHED:
        return _CACHED["absdiff"]
    from concourse import dve_ops
    from concourse.dve_spec import Spec, Src0, Src1, maxx
    from concourse.dve_uop import DveOpSpec

    NAME = "ABSDIFF_ANT"
    for op in dve_ops.OPS:
        if op.name == NAME:
            _CACHED["absdiff"] = op
            return op
    spec = Spec(
        body=maxx(Src0 - Src1, Src1 - Src0),
        reference=lambda in0, in1, s0, s1, imm2: np.abs(
            in0.astype(np.float32) - in1.astype(np.float32)
        ),
    )
    op = dve_ops.DveOp(NAME, spec, subdim=False, uops_sha={})
    dve_ops.OPS.append(op)
    dve_ops.CUSTOM_DVE_SPECS[op.name] = op.spec
    row = dve_ops._CUSTOM_DVE_ROW_BASE + len(dve_ops.OPS) - 1
    dve_ops._SUB_OPCODE_FOR_NAME[op.name] = row
    compiled = DveOpSpec(
        name=NAME,
        opcode=row,
        uops=[_absdiff_uop_1x()],
        uops_2x=[_absdiff_uop_2x()],
        perf_max=1,
        rd1_en=True,
    )
    compiled.validate("v3")
    dve_ops._COMPILE_CACHE[(NAME, "v3")] = compiled
    dve_ops._COMPILE_CACHE[(NAME, "v4")] = compiled
    _CACHED["absdiff"] = op
    return op


# --------------------------------------------------------------------------
# device program
# --------------------------------------------------------------------------

def make_pools(tc, ctx, rep=0):
    sfx = f"_{rep}"
    singles = ctx.enter_context(tc.tile_pool(name="singles" + sfx, bufs=1))
    dbl = ctx.enter_context(tc.tile_pool(name="dbl" + sfx, bufs=2))
    psA = ctx.enter_context(tc.tile_pool(name="psA" + sfx, bufs=2, space="PSUM"))
    psN = ctx.enter_context(tc.tile_pool(name="psN" + sfx, bufs=1, space="PSUM"))
    psO = ctx.enter_context(tc.tile_pool(name="psO" + sfx, bufs=1, space="PSUM"))
    adp = ctx.enter_context(tc.tile_pool(name="adp" + sfx, bufs=6))
    Ep = ctx.enter_context(tc.tile_pool(name="Ep" + sfx, bufs=3))
    return singles, dbl, psA, psN, psO, adp, Ep


def build_body(tc, outs, ins, rep=0, pools=None):
    """Trace the per-core Tile program.

    ins:  xT   [2,128,1024] f16  xT[fh,f,i*32+hwl] = x[i, hw(core,hwl), fh*128+f]
          tw   [2,4,128,128] f16 tw[fh,q,f,b*8+c] = T[fh*128+f,16q+b,c]
          wts  [14,128,32]  f16  0-7: stripe-ones (c-reduce), 8-9: direct
                                 folds per col group, 10-13: half folds (dp,gp)
    outs: o    [64,1024]    f32  o[16q+b, i*32+hwl] = o_b[i, hw(core,hwl), 16q+b]
    """
    from contextlib import ExitStack

    import concourse.mybir as mybir

    nc = tc.nc
    f16 = mybir.dt.float16
    f32 = mybir.dt.float32

    xT_d, tw_d, wts_d = ins["xT"], ins["tw"], ins["wts"]
    o_d = outs["o"]

    with ExitStack() as ctx:
        if pools is None:
            pools = make_pools(tc, ctx, rep)
        singles, dbl, psA, psN, psO, adp, Ep = pools

        # ---- loads (host packs partition-first); tw first and xT split per
        # f-half so stage B's first matmul starts as early as possible.
        tw_t = singles.tile([128, FH * Q * 128], f16, tag="tw")
        nc.sync.dma_start(out=tw_t, in_=tw_d)
        tw_s = [[tw_t[:, (fh * Q + q) * 128:(fh * Q + q + 1) * 128]
                 for q in range(Q)] for fh in range(FH)]
        xT_t = dbl.tile([128, FH * N * HWL], f16, tag="xT")
        nc.sync.dma_start(out=xT_t[:, 0:1024], in_=xT_d[:, 0:1024])
        nc.sync.dma_start(out=xT_t[:, 1024:2048], in_=xT_d[:, 1024:2048])
        xT_s = [xT_t[:, fh * 1024:(fh + 1) * 1024] for fh in range(FH)]
        wts_t = singles.tile([128, 448], f16, tag="wts")
        nc.sync.dma_start(out=wts_t, in_=wts_d)
        ones_s = [wts_t[:, w * 32:(w + 1) * 32] for w in range(8)]
        fold64 = wts_t[:, 256:320]                      # direct, both dp
        hfold64 = [wts_t[:, 320:384], wts_t[:, 384:448]]   # per dp

        # ---- stage B: M2 = (x_slice @ T_q), (b,c)-partition layout, with
        # 512 circularly-padded columns per quarter so every d-stream is one
        # contiguous read: M2v[p, q, k] for k in [0,1536), k>=1024 wraps.
        m2all = dbl.tile([128, Q * 1536], f16, tag="m2")
        M2v = m2all.rearrange("p (q x) -> p q x", q=Q)
        for q in range(Q):
            ps = psA.tile([128, 1024], f32, tag="psA")
            for fh in range(FH):
                for sub in range(2):
                    sl = slice(sub * 512, (sub + 1) * 512)
                    nc.tensor.matmul(
                        ps[:, sl], lhsT=tw_s[fh][q], rhs=xT_s[fh][:, sl],
                        start=(fh == 0), stop=(fh == 1),
                    )
            nc.scalar.copy(out=m2all[:, q * 1536:q * 1536 + 1024], in_=ps[:])
            nc.scalar.copy(out=m2all[:, q * 1536 + 1024:(q + 1) * 1536],
                           in_=ps[:, 0:512])

        # ---- stage C: d-streams ------------------------------------------
        o_ps = psO.tile([128, 1024], f32, tag="oPs")   # rows 0-63 used
        for t in range(8):
            # absdiff for the two d-slots of this group.  Early groups go
            # per-quarter so the DVE starts as soon as stage B's first
            # quarter lands; later groups use one 3D-AP instruction per d.
            ads2 = []
            if t < 2:
                ad0 = adp.tile([128, Q * 1024], f16, tag="ad")
                ad1 = adp.tile([128, Q * 1024], f16, tag="ad")
                ads2 = [ad0, ad1]
                for q in range(Q):
                    for dp in range(2):
                        d = 2 * t + 1 + dp
                        bi = nc.vector._custom_dve(
                            _get_absdiff_op(),
                            out=ads2[dp][:, q * 1024:(q + 1) * 1024],
                            in0=M2v[:, q, d * 32:d * 32 + 1024],
                            in1=M2v[:, q, 0:1024],
                        )
                        bi.ins.perf_max = 1
            else:
                for dp in range(2):
                    d = 2 * t + 1 + dp
                    ad = adp.tile([128, Q * 1024], f16, tag="ad")
                    adv = ad.rearrange("p (q x) -> p q x", q=Q)
                    ln = 1024 if d < 16 else 512
                    bi = nc.vector._custom_dve(
                        _get_absdiff_op(), out=adv[:, :, 0:ln],
                        in0=M2v[:, :, d * 32:d * 32 + ln], in1=M2v[:, :, 0:ln],
                    )
                    bi.ins.perf_max = 1
                    ads2.append(ad)

            # c-reduce: col-tiled stripe-ones matmuls, stripe s = 4*dp+q ->
            # col group s//2, partitions 16s+b = (dp, q, b)
            nrm = psN.tile([128, 1024], f32, tag="nrm")
            for sp in range(2):
                for h in range(2):
                    for g in range(4):
                        s = 2 * g + sp
                        dp, q = s // 4, s % 4
                        if t == 7 and dp == 1 and h == 1:
                            continue     # d=16: columns [512:1024) unused
                        hs = slice(h * 512, (h + 1) * 512)
                        nc.tensor.matmul(
                            nrm[32 * g:32 * g + 32, hs],
                            lhsT=ones_s[s],
                            rhs=ads2[dp][:, q * 1024 + h * 512:
                                          q * 1024 + (h + 1) * 512],
                            start=(sp == 0), stop=(sp == 1),
                            tile_position=(0, 32 * g),
                            skip_group_check=True,
                        )

            # exp(-nrm) -> f16; the last group splits by half and leaves the
            # unused (i>=16) region of the d=16 slot to an early memset so
            # nothing serializes the tail.
            E = Ep.tile([128, 1024], f16, tag="E")
            if t == 7:
                nc.vector.memset(E[64:128, 512:1024], 0.0)
                nc.scalar.activation(
                    out=E[:, 0:512], in_=nrm[:, 0:512],
                    func=mybir.ActivationFunctionType.Exp, scale=-1.0,
                )
                nc.scalar.activation(
                    out=E[0:64, 512:1024], in_=nrm[0:64, 512:1024],
                    func=mybir.ActivationFunctionType.Exp, scale=-1.0,
                )
            else:
                nc.scalar.activation(
                    out=E, in_=nrm[:],
                    func=mybir.ActivationFunctionType.Exp, scale=-1.0,
                )

            # j-sum: direct (row i) + shifted (row i+d) accumulation.  One
            # 128x64 tile per matmul (both col groups at once).
            for h in range(2):
                hs = slice(h * 512, (h + 1) * 512)
                nc.tensor.matmul(
                    o_ps[0:64, hs], lhsT=fold64, rhs=E[:, hs],
                    start=(t == 0), stop=False,
                    tile_position=(0, 0), skip_group_check=True,
                )
            for dp in range(2):
                d = 2 * t + 1 + dp
                if d == 16:
                    segs = [(512, 1024, 0)]
                else:
                    segs = [
                        (32 * d, 512, 0),
                        (512, 1024, 512 - 32 * d),
                        (0, 32 * d, 1024 - 32 * d),
                    ]
                for si, (o0, o1, r0) in enumerate(segs):
                    ln = o1 - o0
                    last = (t == 7 and dp == 1 and si == len(segs) - 1)
                    nc.tensor.matmul(
                        o_ps[0:64, o0:o1],
                        lhsT=hfold64[dp], rhs=E[:, r0:r0 + ln],
                        start=False, stop=last,
                        tile_position=(0, 0), skip_group_check=True,
                    )

        # ---- diagonal (+1) fused into the PSUM drain, then DMA out.  Done
        # per 512-column half so bank A drains while PE finishes bank B.
        o_sb = singles.tile([64, 1024], f32, tag="osb")
        for h in range(2):
            hs = slice(h * 512, (h + 1) * 512)
            nc.scalar.activation(
                out=o_sb[:, hs], in_=o_ps[0:64, hs],
                func=mybir.ActivationFunctionType.Identity, bias=1.0, scale=1.0,
            )
            nc.sync.dma_start(out=o_d[:, hs], in_=o_sb[:, hs])


# --------------------------------------------------------------------------
# host side
# --------------------------------------------------------------------------

def prep_inputs(x, T):
    """Shared (core-independent) device inputs, packed partition-first."""
    xf = np.ascontiguousarray(x.reshape(N, HW, F))
    tw = T.reshape(FH, 128, Q, 16, C).transpose(0, 2, 1, 3, 4)
    tw_in = tw.reshape(FH, Q, 128, 128)
    tw_in = np.ascontiguousarray(
        tw_in.transpose(2, 0, 1, 3).reshape(128, FH * Q * 128)
    ).astype(np.float16)
    wts_in = np.zeros((128, 448), np.float16)
    for s in range(8):
        for b in range(16):
            wts_in[b * 8:(b + 1) * 8, 32 * s + 16 * (s % 2) + b] = 1.0
    for dp in range(2):
        for q in range(4):
            for b in range(16):
                wts_in[64 * dp + 16 * q + b, 256 + 16 * q + b] = 1.0
                wts_in[64 * dp + 16 * q + b, 320 + 64 * dp + 16 * q + b] = 1.0
    return xf, tw_in, wts_in


def core_in_map(xf, tw_in, wts_in, k):
    xs = xf[:, k * HWL:(k + 1) * HWL, :]          # [i, hwl, f]
    xT = xs.transpose(2, 0, 1).reshape(FH, 128, N * HWL)
    xT = np.ascontiguousarray(xT.transpose(1, 0, 2).reshape(128, FH * N * HWL))
    return {"xT": xT.astype(np.float16), "tw": tw_in, "wts": wts_in}


def gather_ob(core_outs):
    """core_outs: list of 8 arrays [64,1024] f32 -> o_b [N,16,16,B]."""
    obs = []
    for res in core_outs:
        v = res.astype(np.float32).reshape(B, N, HWL)   # (16q+b), i, hwl
        obs.append(v.transpose(1, 2, 0))                # i, hwl, b
    return np.concatenate(obs, axis=1).reshape(N, 16, 16, B)


_CACHED = {}


def _get_program(reps=1, loop=None):
    key = ("nc", reps, loop)
    if key in _CACHED:
        return _CACHED[key]
    from contextlib import ExitStack
    import concourse.bacc as bacc
    import concourse.mybir as mybir
    import concourse.tile as tile

    nc = bacc.Bacc("TRN2", target_bir_lowering=False, debug=False,
                   num_devices=CORES)
    f16, f32 = mybir.dt.float16, mybir.dt.float32
    ins = {
        "xT": nc.dram_tensor("xT", [128, FH * N * HWL], f16,
                             kind="ExternalInput").ap(),
        "tw": nc.dram_tensor("tw", [128, FH * Q * 128], f16,
                             kind="ExternalInput").ap(),
        "wts": nc.dram_tensor("wts", [128, 448], f16,
                              kind="ExternalInput").ap(),
    }
    outs = {
        "o": nc.dram_tensor("o", [64, N * HWL], f32, kind="ExternalOutput").ap(),
    }
    with tile.TileContext(nc) as tc:
        if loop:
            with ExitStack() as ctx:
                pools = make_pools(tc, ctx)
                with tc.For_i(0, loop, 1,
                              hint_engines=(mybir.EngineType.PE,
                                            mybir.EngineType.DVE)):
                    build_body(tc, outs, ins, pools=pools)
        else:
            for r in range(reps):
                build_body(tc, outs, ins, rep=r)
    nc.compile()
    _CACHED[key] = nc
    return nc


def kernel(x, T):
    x = np.asarray(x, dtype=np.float32)
    T = np.asarray(T, dtype=np.float32)
    from concourse.bass_utils import run_bass_kernel_spmd

    nc = _get_program()
    xf, tw_in, wts_in = prep_inputs(x, T)
    in_maps = [core_in_map(xf, tw_in, wts_in, k) for k in range(CORES)]
    res = run_bass_kernel_spmd(nc, in_maps, core_ids=list(range(CORES)))
    ob = gather_ob([r["o"] for r in res.results])
    return np.concatenate([x, ob], axis=3)
